# revision 28
# baseline (speedup 1.0000x reference)
"""DGCNN-Lan-PointNet fused kernel for 8 trn2 cores.

Sharding: data-parallel over batch B=8 -> one batch per core. Host prepares
data-dependent gather indices / gathered inputs (FPS + ball-query index math,
pure integer/index work); the device kernel does all matrix math: PointNet++
SA MLPs for 52 objects, LSTM language encoder, DGCNN edge convs (on-device
kNN + gather), pairwise relation MLPs and the object-embedding MLP.
"""

import numpy as np
from contextlib import ExitStack

import concourse.bass as bass
import concourse.bacc as bacc
import concourse.mybir as mybir
import concourse.tile as tile
from concourse.bass_utils import run_bass_kernel_spmd

F32 = mybir.dt.float32
F32R = mybir.dt.float32r
U32 = mybir.dt.uint32

B, N, P, L = 8, 52, 1024, 24
NP1, NS1, R1 = 32, 32, 0.2          # pn_obj sa1
NP2, NS2, R2 = 16, 32, 0.4          # pn_obj sa2
NPR, NSR, RR = 8, 8, 0.4            # rel_pn sa1
NN = N * N                           # 2704 pairs
AI = mybir.AluOpType
AF = mybir.ActivationFunctionType


# ----------------------------------------------------------------------------
# Host-side index/gather prep (exact fp32 replica of reference index math)
# ----------------------------------------------------------------------------

def _fps_batch(xyz, n):
    M = xyz.shape[0]
    idxs = np.zeros((M, n), dtype=np.int64)
    d = ((xyz - xyz[:, :1]) ** 2).sum(-1)
    ar = np.arange(M)
    for t in range(1, n):
        nxt = np.argmax(d, axis=1)
        idxs[:, t] = nxt
        sel = xyz[ar, nxt]
        d = np.minimum(d, ((xyz - sel[:, None]) ** 2).sum(-1))
    return idxs


def _ball_batch(xyz, centers, radius, ns):
    Pn = xyz.shape[1]
    d2 = ((centers[:, :, None] - xyz[:, None]) ** 2).sum(-1)
    key = np.where(d2 < np.float32(radius * radius),
                   np.arange(Pn, dtype=np.int32), np.int32(Pn))
    srt = np.sort(key, axis=2)[:, :, :ns]
    first = srt[:, :, :1]
    first = np.where(first < Pn, first, 0)
    return np.where(srt < Pn, srt, first).astype(np.int64)


def _prep_core(obj_b, lang_b, w):
    xyz = np.ascontiguousarray(obj_b[:, :, :3])
    feat = np.ascontiguousarray(obj_b[:, :, 3:])
    ar = np.arange(N)[:, None, None]

    fi = _fps_batch(xyz, NP1)
    c1 = xyz[np.arange(N)[:, None], fi]
    gi1 = _ball_batch(xyz, c1, R1, NS1)
    g_xyz = xyz[ar, gi1] - c1[:, :, None]
    g_ft = feat[ar, gi1]
    h0 = np.concatenate([g_xyz, g_ft], -1)
    h0s1 = h0.reshape(N, NP1 * NS1, 6).transpose(0, 2, 1)     # [N,6,1024]

    fi2 = _fps_batch(c1, NP2)
    c2 = c1[np.arange(N)[:, None], fi2]
    gi2 = _ball_batch(c1, c2, R2, NS2)
    x2 = c1[ar, gi2] - c2[:, :, None]
    xys2 = x2.reshape(N, NP2 * NS2, 3).transpose(0, 2, 1)     # [N,3,512]
    oh2 = np.zeros((N, NP1, NP2 * NS2), dtype=np.float32)
    flat = gi2.reshape(N, -1)
    oh2[np.arange(N)[:, None], flat, np.arange(NP2 * NS2)[None]] = 1.0

    cr = c1[:, :NPR]
    gir = _ball_batch(xyz, cr, RR, NSR)
    hr = xyz[ar, gir] - cr[:, :, None]
    h0r = hr.reshape(N, NPR * NSR, 3).transpose(1, 2, 0)      # -> build below

    d = {
        "h0s1": np.ascontiguousarray(h0s1, np.float32),
        "xys2": np.ascontiguousarray(xys2, np.float32),
        "oh2": oh2,
        # [3, N*64] / [3, N*8] / [3, N*16], c-major outer, object-major free
        "h0r": np.ascontiguousarray(
            hr.reshape(N, NPR * NSR, 3).transpose(2, 0, 1).reshape(3, -1),
            np.float32),
        "xy1r": np.ascontiguousarray(
            cr.transpose(2, 0, 1).reshape(3, -1), np.float32),
        "xy2g": np.ascontiguousarray(
            c2.transpose(2, 0, 1).reshape(3, -1), np.float32),
    }
    emb = np.asarray(w["plang"]["emb"], np.float32)[lang_b]   # [L,64]
    d["xemb"] = np.ascontiguousarray(emb.T, np.float32)
    madd = np.where(lang_b != 0, 0.0, -2e9).astype(np.float32)
    d["madd"] = np.ascontiguousarray(
        np.broadcast_to(madd[None], (128, L)), np.float32)
    return d


def _weights_map(w):
    m = {}
    s1 = w["prp"]["sa1"]
    m["w_r1"], m["b_r1"] = s1[0]["w"][:3], s1[0]["b"][:, None]
    m["w_r2"], m["b_r2"] = s1[1]["w"], s1[1]["b"][:, None]
    m["w_r3"], m["b_r3"] = s1[2]["w"], s1[2]["b"][:, None]
    rg = w["prp"]["glob"]
    m["w_rg1x"], m["w_rg1f"] = rg[0]["w"][:3], rg[0]["w"][3:]
    m["b_rg1"] = rg[0]["b"][:, None]
    m["w_rg2"], m["b_rg2"] = rg[1]["w"], rg[1]["b"][:, None]
    m["w_rg3"], m["b_rg3"] = rg[2]["w"], rg[2]["b"][:, None]
    o1 = w["pop"]["sa1"]
    m["w_s1_1"], m["b_s1_1"] = o1[0]["w"], o1[0]["b"][:, None]
    m["w_s1_2"], m["b_s1_2"] = o1[1]["w"], o1[1]["b"][:, None]
    m["w_s1_3"], m["b_s1_3"] = o1[2]["w"], o1[2]["b"][:, None]
    o2 = w["pop"]["sa2"]
    m["w_s2_1x"], m["w_s2_1f"] = o2[0]["w"][:3], o2[0]["w"][3:]
    m["b_s2_1"] = o2[0]["b"][:, None]
    m["w_s2_2"], m["b_s2_2"] = o2[1]["w"], o2[1]["b"][:, None]
    m["w_s2_3"], m["b_s2_3"] = o2[2]["w"], o2[2]["b"][:, None]
    og = w["pop"]["glob"]
    m["w_g1x"], m["w_g1f"] = og[0]["w"][:3], og[0]["w"][3:]
    m["b_g1"] = og[0]["b"][:, None]
    m["w_g2"], m["b_g2"] = og[1]["w"], og[1]["b"][:, None]
    m["w_g3"], m["b_g3"] = og[2]["w"], og[2]["b"][:, None]
    m["w_lp"], m["b_lp"] = w["plang"]["proj"]["w"], w["plang"]["proj"]["b"][:, None]
    m["w_li"], m["w_lh"] = w["plang"]["wi"], w["plang"]["wh"]
    m["b_l"] = w["plang"]["b"][:, None]
    for i, lpw in enumerate(w["pdg"]["edge"]):
        m[f"w_e{i + 1}"], m[f"b_e{i + 1}"] = lpw["w"], lpw["b"][:, None]
    m["w_f"], m["b_f"] = w["pdg"]["final"]["w"], w["pdg"]["final"]["b"][:, None]
    for i, lpw in enumerate(w["pmlp"]):
        m[f"w_m{i + 1}"], m[f"b_m{i + 1}"] = lpw["w"], lpw["b"][:, None]
    m["w_p1"], m["b_p1"] = w["prel"]["l1"]["w"], w["prel"]["l1"]["b"][:, None]
    m["w_p2"], m["b_p2"] = w["prel"]["l2"]["w"], w["prel"]["l2"]["b"][:, None]
    m["w_q1"], m["b_q1"] = w["pmrel"]["l1"]["w"], w["pmrel"]["l1"]["b"][:, None]
    m["w_q2"], m["b_q2"] = w["pmrel"]["l2"]["w"], w["pmrel"]["l2"]["b"][:, None]
    return {k: np.ascontiguousarray(np.asarray(v, np.float32))
            for k, v in m.items()}



WSHAPES = dict(
    w_r1=(3, 32), b_r1=(32, 1), w_r2=(32, 32), b_r2=(32, 1),
    w_r3=(32, 64), b_r3=(64, 1),
    w_rg1x=(3, 64), w_rg1f=(64, 64), b_rg1=(64, 1),
    w_rg2=(64, 64), b_rg2=(64, 1), w_rg3=(64, 128), b_rg3=(128, 1),
    w_s1_1=(6, 64), b_s1_1=(64, 1), w_s1_2=(64, 64), b_s1_2=(64, 1),
    w_s1_3=(64, 128), b_s1_3=(128, 1),
    w_s2_1x=(3, 128), w_s2_1f=(128, 128), b_s2_1=(128, 1),
    w_s2_2=(128, 128), b_s2_2=(128, 1), w_s2_3=(128, 256), b_s2_3=(256, 1),
    w_g1x=(3, 256), w_g1f=(256, 256), b_g1=(256, 1),
    w_g2=(256, 512), b_g2=(512, 1), w_g3=(512, 128), b_g3=(128, 1),
    w_lp=(64, 64), b_lp=(64, 1), w_li=(64, 512), w_lh=(128, 512),
    b_l=(512, 1),
    w_e1=(512, 128), b_e1=(128, 1), w_e2=(256, 128), b_e2=(128, 1),
    w_e3=(256, 128), b_e3=(128, 1), w_e4=(256, 128), b_e4=(128, 1),
    w_f=(512, 128), b_f=(128, 1),
    w_m1=(128, 128), b_m1=(128, 1), w_m2=(128, 256), b_m2=(256, 1),
    w_m3=(256, 607), b_m3=(607, 1),
    w_p1=(256, 128), b_p1=(128, 1), w_p2=(128, 128), b_p2=(128, 1),
    w_q1=(128, 128), b_q1=(128, 1), w_q2=(128, 128), b_q2=(128, 1),
)


def _wlayout():
    """[(key, chunk_idx, rows, cols, col_off)] packing into [128, TOT]."""
    out, off = [], 0
    for k, (rows, cols) in WSHAPES.items():
        for ci, r0 in enumerate(range(0, rows, 128)):
            r1 = min(rows, r0 + 128)
            out.append((k, ci, r1 - r0, cols, off))
            off += cols
    return out, off


def _pack_weights(wm):
    lay, tot = _wlayout()
    arr = np.zeros((128, tot), np.float32)
    for k, ci, r, c, off in lay:
        arr[:r, off:off + c] = wm[k][ci * 128:ci * 128 + r]
    return arr


# ----------------------------------------------------------------------------
# Device kernel
# ----------------------------------------------------------------------------

def build_bass():
    nc = bacc.Bacc(None)
    ctx = ExitStack()

    def par(name, shape, out=False):
        return nc.declare_dram_parameter(name, list(shape), F32, isOutput=out)

    h0s1 = par("h0s1", (N, 6, 1024))
    xys2 = par("xys2", (N, 3, 512))
    oh2 = par("oh2", (N, 32, 512))
    h0r = par("h0r", (3, N * 64))
    xy1r = par("xy1r", (3, N * NPR))
    xy2g = par("xy2g", (3, N * NP2))
    xemb = par("xemb", (64, L))
    madd = par("madd", (128, L))

    lay, wtot = _wlayout()
    wpack = par("wpack", (128, wtot))
    wshapes = WSHAPES
    objT = par("objT", (607, N), out=True)
    dbg1 = par("dbg1", (52, N), out=True)    # nd2 layer1
    dbg2 = par("dbg2", (52, 8), out=True)    # mif layer1
    dbg3 = par("dbg3", (128, N), out=True)   # hn0
    relT = par("relT", (128, NN), out=True)
    mrelT = par("mrelT", (128, NN), out=True)

    with tile.TileContext(nc) as tc:
        cp = ctx.enter_context(tc.tile_pool(name="const", bufs=1))
        wp = ctx.enter_context(tc.tile_pool(name="wts", bufs=1))
        big = ctx.enter_context(tc.tile_pool(name="big", bufs=1))
        lp = ctx.enter_context(tc.tile_pool(name="loop", bufs=3))
        ep = ctx.enter_context(tc.tile_pool(name="ev", bufs=2))
        sp = ctx.enter_context(tc.tile_pool(name="small", bufs=2))
        rp2 = ctx.enter_context(tc.tile_pool(name="rp2", bufs=2))
        rlb = ctx.enter_context(tc.tile_pool(name="rlb", bufs=2))
        pp = ctx.enter_context(tc.tile_pool(name="ps", bufs=7, space="PSUM"))

        def psum(m, n):
            return pp.tile([m, n], F32, tag="ps", name="ps")

        def mm(ps, pairs, rdt=F32R, n0=0):
            # ps: psum AP [M, n<=512]; pairs: K-chunks (lhsT, rhs_full)
            nsz = ps.shape[-1]
            dt = F32  # fp32r needs rounded producers; revisit
            for ki, (lh, rh) in enumerate(pairs):
                nc.tensor.matmul(ps, lh.bitcast(dt),
                                 rh[:, n0:n0 + nsz].bitcast(dt),
                                 start=(ki == 0), stop=(ki == len(pairs) - 1))

        def relu_b(eng, out, src, bcol):
            if eng == "act":
                nc.scalar.activation(out, src, AF.Relu, bias=bcol)
            else:
                nc.vector.tensor_scalar(out, src, bcol, 0.0, AI.add, AI.max)

        def layer(pairs, out_tile, M, Ntot, bcol, mode, nmax=512, rot=0,
                  rdt=F32R):
            # full layer: loop N-chunks: matmul -> evict into out_tile
            # mode: 'relu' | 'bias' | None
            ci = rot
            for n0 in range(0, Ntot, nmax):
                n1 = min(Ntot, n0 + nmax)
                ps = psum(M, n1 - n0)
                mm(ps[:], pairs, n0=n0, rdt=rdt)
                dst = out_tile[:, n0:n1]
                if mode == "relu":
                    relu_b("act" if ci % 2 == 0 else "dve", dst, ps[:], bcol)
                elif mode == "bias":
                    nc.scalar.activation(dst, ps[:], AF.Identity, bias=bcol)
                else:
                    nc.scalar.copy(dst, ps[:])
                ci += 1

        # --- all weights in one packed tile (one DMA, one wait sem)
        WP = wp.tile([128, wtot], F32, tag="wpack", name="WP")
        nc.sync.dma_start(out=WP[:], in_=wpack[:])
        W = {}
        for k, ci, r, c, off in lay:
            ap = WP[0:r, off:off + c]
            if wshapes[k][0] <= 128:
                W[k] = ap
            else:
                W.setdefault(k, []).append(ap)

        from concourse.masks import make_identity
        idt = cp.tile([128, 128], F32, tag="idt")
        make_identity(nc, idt[:])
        iota52 = cp.tile([52, 52], F32, tag="iota")
        nc.gpsimd.iota(iota52[:], pattern=[[1, 52]], base=0,
                       channel_multiplier=0,
                       allow_small_or_imprecise_dtypes=True)
        zero52 = cp.tile([128, 52], F32, tag="z52")
        nc.vector.memset(zero52[:], 0.0)
        ones1 = cp.tile([128, 1], F32, tag="o1")
        nc.vector.memset(ones1[:], 1.0)

        # =========== pn_obj sa1 (per object) ===========
        f1T = big.tile([128, N * NP1], F32, tag="f1T")
        for o in range(N):
            h0 = lp.tile([6, 1024], F32, tag="h0")
            nc.sync.dma_start(out=h0[:], in_=h0s1[o])
            a1 = ep.tile([64, 1024], F32, tag="a1")
            layer([(W["w_s1_1"][:], h0[:])], a1, 64, 1024, W["b_s1_1"][:],
                  "relu", rot=o)
            a2 = ep.tile([64, 1024], F32, tag="a2")
            layer([(W["w_s1_2"][:], a1[:])], a2, 64, 1024, W["b_s1_2"][:],
                  "relu", rot=o + 1)
            for half in range(2):
                ps = psum(128, 512)
                mm(ps[:], [(W["w_s1_3"][:], a2[:])], n0=half * 512)
                nc.vector.tensor_reduce(
                    f1T[:, o * NP1 + half * 16:o * NP1 + (half + 1) * 16],
                    ps[:].rearrange("p (c s) -> p c s", s=NS1),
                    axis=mybir.AxisListType.X, op=AI.max)
            relu_b("act", f1T[:, o * NP1:(o + 1) * NP1],
                   f1T[:, o * NP1:(o + 1) * NP1], W["b_s1_3"][:])

        # =========== pn_obj sa2 (per object) ===========
        f2aT = big.tile([128, N * NP2], F32, tag="f2aT")
        f2bT = big.tile([128, N * NP2], F32, tag="f2bT")
        for o in range(N):
            pst = psum(32, 128)
            nc.tensor.matmul(pst[:], f1T[:, o * NP1:(o + 1) * NP1],
                             idt[:], is_transpose=True, start=True, stop=True)
            f1row = lp.tile([32, 128], F32, tag="f1row")
            nc.scalar.copy(f1row[:], pst[:])
            oh = lp.tile([32, 512], F32, tag="oh")
            nc.sync.dma_start(out=oh[:], in_=oh2[o])
            g2 = ep.tile([128, 512], F32, tag="g2")
            layer([(f1row[:], oh[:])], g2, 128, 512, None, None, rdt=F32)
            x2 = lp.tile([3, 512], F32, tag="x2")
            nc.sync.dma_start(out=x2[:], in_=xys2[o])
            s2a = ep.tile([128, 512], F32, tag="s2a")
            layer([(W["w_s2_1x"][:], x2[:]), (W["w_s2_1f"][:], g2[:])],
                  s2a, 128, 512, W["b_s2_1"][:], "relu", rot=o)
            s2b = ep.tile([128, 512], F32, tag="s2b")
            layer([(W["w_s2_2"][:], s2a[:])], s2b, 128, 512, W["b_s2_2"][:],
                  "relu", rot=o + 1)
            for half in range(2):
                ps = psum(128, 512)
                mm(ps[:], [(W["w_s2_3"][:, half * 128:(half + 1) * 128], s2b[:])])
                dst = (f2aT if half == 0 else f2bT)[:, o * NP2:(o + 1) * NP2]
                nc.vector.tensor_reduce(
                    dst, ps[:].rearrange("p (c s) -> p c s", s=NS2),
                    axis=mybir.AxisListType.X, op=AI.max)
                relu_b("act", dst, dst, W["b_s2_3"][half][:])

        # =========== pn_obj global SA (batched) ===========
        x2g = big.tile([3, N * NP2], F32, tag="x2g")
        nc.sync.dma_start(out=x2g[:], in_=xy2g[:])
        ga = big.tile([128, N * NP2], F32, tag="ga")
        gb = big.tile([128, N * NP2], F32, tag="gb")
        layer([(W["w_g1x"][:, 0:128], x2g[:]),
               (W["w_g1f"][0][:, 0:128], f2aT[:]),
               (W["w_g1f"][1][:, 0:128], f2bT[:])],
              ga, 128, N * NP2, W["b_g1"][0][:], "relu", nmax=416)
        layer([(W["w_g1x"][:, 128:256], x2g[:]),
               (W["w_g1f"][0][:, 128:256], f2aT[:]),
               (W["w_g1f"][1][:, 128:256], f2bT[:])],
              gb, 128, N * NP2, W["b_g1"][1][:], "relu", nmax=416, rot=1)
        gc = []
        for mi in range(4):
            t = big.tile([128, N * NP2], F32, tag=f"gc{mi}")
            layer([(W["w_g2"][0][:, mi * 128:(mi + 1) * 128], ga[:]),
                   (W["w_g2"][1][:, mi * 128:(mi + 1) * 128], gb[:])],
                  t, 128, N * NP2, W["b_g2"][mi][:],
                  "relu", nmax=416, rot=mi)
            gc.append(t)
        ofT = big.tile([128, N], F32, tag="ofT")
        for half in range(2):
            ps = psum(128, 416)
            mm(ps[:], [(W["w_g3"][mi][:], gc[mi][:])
                       for mi in range(4)], n0=half * 416)
            nc.vector.tensor_reduce(
                ofT[:, half * 26:(half + 1) * 26],
                ps[:].rearrange("p (n c) -> p n c", c=NP2),
                axis=mybir.AxisListType.X, op=AI.max)
        relu_b("act", ofT[:], ofT[:], W["b_g3"][:])

        # =========== rel_pn branch (batched) ===========
        h0rt = rp2.tile([3, N * 64], F32, tag="rp", name="h0rt")
        nc.sync.dma_start(out=h0rt[:], in_=h0r[:])
        r1 = rp2.tile([32, N * 64], F32, tag="rp", name="r1")
        layer([(W["w_r1"][:], h0rt[:])], r1, 32, N * 64, W["b_r1"][:], "relu")
        r2 = rp2.tile([32, N * 64], F32, tag="rp", name="r2")
        layer([(W["w_r2"][:], r1[:])], r2, 32, N * 64, W["b_r2"][:], "relu",
              rot=1)
        frT = big.tile([64, N * NPR], F32, tag="frT")
        for qi in range(0, N * 64, 512):
            nsz = min(512, N * 64 - qi)
            ps = psum(64, nsz)
            mm(ps[:], [(W["w_r3"][:], r2[:])], n0=qi)
            nc.vector.tensor_reduce(
                frT[:, qi // NSR:(qi + nsz) // NSR],
                ps[:].rearrange("p (c s) -> p c s", s=NSR),
                axis=mybir.AxisListType.X, op=AI.max)
        relu_b("act", frT[:], frT[:], W["b_r3"][:])

        x1r = big.tile([3, N * NPR], F32, tag="x1r")
        nc.sync.dma_start(out=x1r[:], in_=xy1r[:])
        q1 = big.tile([64, N * NPR], F32, tag="q1")
        layer([(W["w_rg1x"][:], x1r[:]), (W["w_rg1f"][:], frT[:])],
              q1, 64, N * NPR, W["b_rg1"][:], "relu", nmax=416)
        q2 = big.tile([64, N * NPR], F32, tag="q2")
        layer([(W["w_rg2"][:], q1[:])], q2, 64, N * NPR, W["b_rg2"][:],
              "relu", nmax=416, rot=1)
        rpT = big.tile([128, N], F32, tag="rpT")
        ps = psum(128, 416)
        mm(ps[:], [(W["w_rg3"][:], q2[:])])
        nc.vector.tensor_reduce(
            rpT[:], ps[:].rearrange("p (n s) -> p n s", s=NPR),
            axis=mybir.AxisListType.X, op=AI.max)
        relu_b("act", rpT[:], rpT[:], W["b_rg3"][:])

        # =========== LSTM (this core's batch) ===========
        xe = cp.tile([64, L], F32, tag="xe")
        nc.sync.dma_start(out=xe[:], in_=xemb[:])
        mad = cp.tile([128, L], F32, tag="mad")
        nc.sync.dma_start(out=mad[:], in_=madd[:])
        xT = cp.tile([64, L], F32, tag="xT")
        ps = psum(64, L)
        mm(ps[:], [(W["w_lp"][:], xe[:])])
        relu_b("act", xT[:], ps[:], W["b_lp"][:])
        cst = cp.tile([128, 1], F32, tag="cst")
        nc.vector.memset(cst[:], 0.0)
        hs0 = cp.tile([128, 1], F32, tag="hs0")
        nc.vector.memset(hs0[:], 0.0)
        hsT = cp.tile([128, L], F32, tag="hsT")
        hprev = hs0[:]
        for t in range(L):
            psz = psum(128, 4)
            for mi in range(4):
                nc.tensor.matmul(psz[:, mi:mi + 1],
                                 W["w_li"][:, mi * 128:(mi + 1) * 128],
                                 xT[:, t:t + 1], start=True, stop=False)
                nc.tensor.matmul(psz[:, mi:mi + 1],
                                 W["w_lh"][:, mi * 128:(mi + 1) * 128],
                                 hprev, start=False, stop=True)
            gi_ = sp.tile([128, 1], F32, tag="l_i")
            gf_ = sp.tile([128, 1], F32, tag="l_f")
            gg_ = sp.tile([128, 1], F32, tag="l_g")
            go_ = sp.tile([128, 1], F32, tag="l_o")
            nc.scalar.activation(gi_[:], psz[:, 0:1], AF.Sigmoid, bias=W["b_l"][0][:])
            nc.scalar.activation(gf_[:], psz[:, 1:2], AF.Sigmoid, bias=W["b_l"][1][:])
            nc.scalar.activation(gg_[:], psz[:, 2:3], AF.Tanh, bias=W["b_l"][2][:])
            nc.scalar.activation(go_[:], psz[:, 3:4], AF.Sigmoid, bias=W["b_l"][3][:])
            t1 = sp.tile([128, 1], F32, tag="l_t1")
            nc.vector.tensor_tensor(t1[:], gf_[:], cst[:], AI.mult)
            t2 = sp.tile([128, 1], F32, tag="l_t2")
            nc.vector.tensor_tensor(t2[:], gi_[:], gg_[:], AI.mult)
            cst = sp.tile([128, 1], F32, tag="l_c")
            nc.vector.tensor_tensor(cst[:], t1[:], t2[:], AI.add)
            tc_ = sp.tile([128, 1], F32, tag="l_tc")
            nc.scalar.activation(tc_[:], cst[:], AF.Tanh)
            nc.vector.tensor_tensor(hsT[:, t:t + 1], go_[:], tc_[:], AI.mult)
            hprev = hsT[:, t:t + 1]
        hm = cp.tile([128, L], F32, tag="hm")
        nc.vector.tensor_tensor(hm[:], hsT[:], mad[:], AI.add)
        lf = cp.tile([128, 1], F32, tag="lf")
        nc.vector.tensor_reduce(lf[:], hm[:], axis=mybir.AxisListType.X, op=AI.max)
        lfb = big.tile([128, N], F32, tag="lfb")
        nc.vector.tensor_scalar(lfb[:], zero52[:], lf[:], None, AI.add)

        # =========== DGCNN (on-device kNN, k=3) ===========
        onesr = cp.tile([1, N], F32, tag="onesr")
        nc.vector.memset(onesr[:], 1.0)
        mpsq = cp.tile([1, N], F32, tag="mpsq")
        feats = []
        hcur = [rpT, lfb]
        for li in range(4):
            nch = len(hcur)
            pspq = psum(1, N)
            sqs = []
            for c in range(nch):
                sq = sp.tile([128, N], F32, tag=f"sq{c}")
                nc.scalar.activation(sq[:], hcur[c][:], AF.Square)
                sqs.append(sq)
            mm(pspq[:], [(ones1[:], s[:]) for s in sqs], rdt=F32)
            nc.scalar.activation(mpsq[:], pspq[:], AF.Copy, scale=-1.0)
            h2s = []
            for c in range(nch):
                h2 = sp.tile([128, N], F32, tag=f"h2_{c}")
                nc.scalar.activation(h2[:], hcur[c][:], AF.Copy, scale=2.0)
                h2s.append(h2)
            psd2 = psum(52, N)
            mm(psd2[:], [(hcur[c][:], h2s[c][:]) for c in range(nch)]
               + [(mpsq[:], onesr[:]), (onesr[:], mpsq[:])], rdt=F32)
            nd2 = sp.tile([52, N], F32, tag="nd2")
            nc.scalar.copy(nd2[:], psd2[:])
            mx = sp.tile([52, 8], F32, tag="mx")
            nc.vector.max(mx[:], nd2[:])
            mi_ = sp.tile([52, 8], U32, tag="mi")
            nc.vector.max_index(mi_[:], mx[:], nd2[:])
            mif = sp.tile([52, 8], F32, tag="mif")
            nc.vector.tensor_copy(mif[:], mi_[:])
            if li == 0:
                nc.sync.dma_start(out=dbg1[:], in_=nd2[:])
                nc.sync.dma_start(out=dbg2[:], in_=mif[:])

            hrow = []
            for c in range(nch):
                pst = psum(52, 128)
                nc.tensor.matmul(pst[:], hcur[c][:], idt[:],
                                 is_transpose=True, start=True, stop=True)
                hr_ = sp.tile([52, 128], F32, tag=f"hr{c}")
                nc.scalar.copy(hr_[:], pst[:])
                hrow.append(hr_)

            wE = W[f"w_e{li + 1}"]
            acc = sp.tile([128, N], F32, tag="eacc")
            for s in range(3):
                ohT = sp.tile([52, N], F32, tag="ohT")
                nc.vector.tensor_scalar(ohT[:], iota52[:], mif[:, s:s + 1],
                                        None, AI.is_equal)
                psoh = psum(52, N)
                nc.tensor.matmul(psoh[:], ohT[:], idt[0:52, 0:52],
                                 is_transpose=True, start=True, stop=True)
                ohJ = sp.tile([52, N], F32, tag="ohJ")
                nc.scalar.copy(ohJ[:], psoh[:])
                difs = []
                for c in range(nch):
                    psx = psum(128, N)
                    mm(psx[:], [(hrow[c][:], ohJ[:])], rdt=F32)
                    dif = sp.tile([128, N], F32, tag=f"dif{c}")
                    nc.vector.tensor_tensor(dif[:], psx[:], hcur[c][:],
                                            AI.subtract)
                    difs.append(dif)
                pse = psum(128, N)
                pairs = [(wE[c][:], hcur[c][:])
                         for c in range(nch)]
                pairs += [(wE[nch + c][:], difs[c][:])
                          for c in range(nch)]
                mm(pse[:], pairs, rdt=F32)
                if s == 0:
                    nc.scalar.copy(acc[:], pse[:])
                else:
                    nc.vector.tensor_tensor(acc[:], acc[:], pse[:], AI.max)
            hb = sp.tile([128, N], F32, tag="hb")
            nc.scalar.activation(hb[:], acc[:], AF.Identity,
                                 bias=W[f"b_e{li + 1}"][:])
            hn = big.tile([128, N], F32, tag=f"hn{li}")
            nc.vector.scalar_tensor_tensor(hn[:], hb[:], 0.2, hb[:],
                                           AI.mult, AI.max)
            if li == 0:
                nc.sync.dma_start(out=dbg3[:], in_=hn[:])
            feats.append(hn)
            hcur = [hn]

        gT = big.tile([128, N], F32, tag="gT")
        ps = psum(128, N)
        mm(ps[:], [(W["w_f"][c][:], feats[c][:])
                   for c in range(4)], rdt=F32)
        gb_ = sp.tile([128, N], F32, tag="gb_")
        nc.scalar.activation(gb_[:], ps[:], AF.Identity, bias=W["b_f"][:])
        nc.vector.scalar_tensor_tensor(gT[:], gb_[:], 0.2, gb_[:],
                                       AI.mult, AI.max)

        # =========== relation MLPs ===========
        rf = big.tile([128, N], F32, tag="rf")
        nc.scalar.activation(rf[:], gT[:], AF.Relu)

        rel1r = rlb.tile([128, NN], F32, tag="relb", name="rel1r")
        relor = rlb.tile([128, NN], F32, tag="relb", name="relor")
        mr1r = rlb.tile([128, NN], F32, tag="relb", name="mr1r")

        # layer 1 from on-the-fly expanded (f_i, f_j) chunks
        for bi in range(0, N, 8):
            nb = min(8, N - bi)              # i-blocks in this chunk
            u1 = sp.tile([128, nb * N], F32, tag="u1")
            u2 = sp.tile([128, nb * N], F32, tag="u2")
            for k in range(nb):
                i = bi + k
                nc.vector.tensor_scalar(u1[:, k * N:(k + 1) * N], zero52[:],
                                        rf[:, i:i + 1], None, AI.add)
                nc.scalar.copy(u2[:, k * N:(k + 1) * N], rf[:])
            ps = psum(128, nb * N)
            mm(ps[:], [(W["w_p1"][0][:], u1[:]),
                       (W["w_p1"][1][:], u2[:])])
            relu_b("dve", rel1r[:, bi * N:(bi + nb) * N], ps[:], W["b_p1"][:])

        def rel_layer(wk, bk, src, relu_dst, out_dram):
            for n0 in range(0, NN, 416):
                nsz = min(416, NN - n0)
                ps = psum(128, nsz)
                mm(ps[:], [(W[wk][:], src[:])], n0=n0)
                if relu_dst is not None:
                    relu_b("dve", relu_dst[:, n0:n0 + nsz], ps[:], W[bk][:])
                if out_dram is not None:
                    ot = sp.tile([128, nsz], F32, tag="rel_o")
                    nc.scalar.activation(ot[:], ps[:], AF.Identity,
                                         bias=W[bk][:])
                    nc.sync.dma_start(out=out_dram[:, n0:n0 + nsz], in_=ot[:])

        rel_layer("w_p2", "b_p2", rel1r, relor, relT)
        rel_layer("w_q1", "b_q1", relor, mr1r, None)
        rel_layer("w_q2", "b_q2", mr1r, None, mrelT)

        # =========== object-embedding MLP ===========
        m1 = sp.tile([128, N], F32, tag="m1")
        ps = psum(128, N)
        mm(ps[:], [(W["w_m1"][:], ofT[:])], rdt=F32)
        relu_b("act", m1[:], ps[:], W["b_m1"][:])
        m2t = []
        for mi in range(2):
            ps = psum(128, N)
            mm(ps[:], [(W["w_m2"][:, mi * 128:(mi + 1) * 128], m1[:])], rdt=F32)
            t = sp.tile([128, N], F32, tag=f"m2_{mi}")
            relu_b("act" if mi == 0 else "dve", t[:], ps[:],
                   W["b_m2"][mi][:])
            m2t.append(t)
        for mi in range(5):
            m0, m1c = mi * 128, min(607, (mi + 1) * 128)
            ps = psum(m1c - m0, N)
            mm(ps[:], [(W["w_m3"][0][:, m0:m1c], m2t[0][:]),
                       (W["w_m3"][1][:, m0:m1c], m2t[1][:])], rdt=F32)
            ot = sp.tile([m1c - m0, N], F32, tag="obj_o")
            nc.scalar.activation(ot[:], ps[:], AF.Identity, bias=W["b_m3"][mi][:])
            nc.sync.dma_start(out=objT[m0:m1c, :], in_=ot[:])

        ctx.close()
    if not nc.is_finalized():
        nc.finalize()
    return nc


_NC_CACHE = {}


def _to_np(x):
    if isinstance(x, dict):
        return {k: _to_np(v) for k, v in x.items()}
    if isinstance(x, (list, tuple)):
        return [_to_np(v) for v in x]
    return np.asarray(x)


def kernel(**inputs):
    objects = np.asarray(inputs["objects"], np.float32)
    lang = np.asarray(inputs["lang"])
    w = {k: _to_np(inputs[k]) for k in
         ("prp", "pop", "pdg", "plang", "pmlp", "prel", "pmrel")}
    wm = _weights_map(w)
    wp = _pack_weights(wm)

    in_maps = []
    for b in range(B):
        d = _prep_core(objects[b], lang[b], w)
        d["wpack"] = wp
        in_maps.append(d)

    if "nc" not in _NC_CACHE:
        _NC_CACHE["nc"] = build_bass()
    nc = _NC_CACHE["nc"]
    res = run_bass_kernel_spmd(nc, in_maps, list(range(B))).results

    obj = np.stack([res[b]["objT"].T for b in range(B)], 1)
    rel = np.stack([res[b]["relT"].T.reshape(N, N, 128) for b in range(B)], 2)
    mrel = np.stack([res[b]["mrelT"].T.reshape(N, N, 128) for b in range(B)], 2)
    return obj, rel, mrel


# revision 29
# speedup vs baseline: 1.1263x; 1.1263x over previous
"""DGCNN-Lan-PointNet fused kernel for 8 trn2 cores.

Sharding: data-parallel over batch B=8 -> one batch per core. Host prepares
data-dependent gather indices / gathered inputs (FPS + ball-query index math,
pure integer/index work); the device kernel does all matrix math: PointNet++
SA MLPs for 52 objects, LSTM language encoder, DGCNN edge convs (on-device
kNN + gather), pairwise relation MLPs and the object-embedding MLP.
"""

import numpy as np
from contextlib import ExitStack

import concourse.bass as bass
import concourse.bacc as bacc
import concourse.mybir as mybir
import concourse.tile as tile
from concourse.bass_utils import run_bass_kernel_spmd

F32 = mybir.dt.float32
F32R = mybir.dt.float32r
U32 = mybir.dt.uint32

B, N, P, L = 8, 52, 1024, 24
NP1, NS1, R1 = 32, 32, 0.2          # pn_obj sa1
NP2, NS2, R2 = 16, 32, 0.4          # pn_obj sa2
NPR, NSR, RR = 8, 8, 0.4            # rel_pn sa1
NN = N * N                           # 2704 pairs
AI = mybir.AluOpType
AF = mybir.ActivationFunctionType


# ----------------------------------------------------------------------------
# Host-side index/gather prep (exact fp32 replica of reference index math)
# ----------------------------------------------------------------------------

def _fps_batch(xyz, n):
    M = xyz.shape[0]
    idxs = np.zeros((M, n), dtype=np.int64)
    d = ((xyz - xyz[:, :1]) ** 2).sum(-1)
    ar = np.arange(M)
    for t in range(1, n):
        nxt = np.argmax(d, axis=1)
        idxs[:, t] = nxt
        sel = xyz[ar, nxt]
        d = np.minimum(d, ((xyz - sel[:, None]) ** 2).sum(-1))
    return idxs


def _ball_batch(xyz, centers, radius, ns):
    Pn = xyz.shape[1]
    d2 = ((centers[:, :, None] - xyz[:, None]) ** 2).sum(-1)
    key = np.where(d2 < np.float32(radius * radius),
                   np.arange(Pn, dtype=np.int32), np.int32(Pn))
    srt = np.sort(key, axis=2)[:, :, :ns]
    first = srt[:, :, :1]
    first = np.where(first < Pn, first, 0)
    return np.where(srt < Pn, srt, first).astype(np.int64)


def _prep_core(obj_b, lang_b, w):
    xyz = np.ascontiguousarray(obj_b[:, :, :3])
    feat = np.ascontiguousarray(obj_b[:, :, 3:])
    ar = np.arange(N)[:, None, None]

    fi = _fps_batch(xyz, NP1)
    c1 = xyz[np.arange(N)[:, None], fi]
    gi1 = _ball_batch(xyz, c1, R1, NS1)
    g_xyz = xyz[ar, gi1] - c1[:, :, None]
    g_ft = feat[ar, gi1]
    h0 = np.concatenate([g_xyz, g_ft], -1)
    h0s1 = h0.reshape(N, NP1 * NS1, 6).transpose(0, 2, 1)     # [N,6,1024]

    fi2 = _fps_batch(c1, NP2)
    c2 = c1[np.arange(N)[:, None], fi2]
    gi2 = _ball_batch(c1, c2, R2, NS2)
    x2 = c1[ar, gi2] - c2[:, :, None]
    xys2 = x2.reshape(N, NP2 * NS2, 3).transpose(0, 2, 1)     # [N,3,512]
    oh2 = np.zeros((N, NP1, NP2 * NS2), dtype=np.float32)
    flat = gi2.reshape(N, -1)
    oh2[np.arange(N)[:, None], flat, np.arange(NP2 * NS2)[None]] = 1.0

    cr = c1[:, :NPR]
    gir = _ball_batch(xyz, cr, RR, NSR)
    hr = xyz[ar, gir] - cr[:, :, None]
    h0r = hr.reshape(N, NPR * NSR, 3).transpose(1, 2, 0)      # -> build below

    d = {
        "h0s1": np.ascontiguousarray(h0s1, np.float32),
        "xys2": np.ascontiguousarray(xys2, np.float32),
        "oh2": oh2,
        # [3, N*64] / [3, N*8] / [3, N*16], c-major outer, object-major free
        "h0r": np.ascontiguousarray(
            hr.reshape(N, NPR * NSR, 3).transpose(2, 0, 1).reshape(3, -1),
            np.float32),
        "xy1r": np.ascontiguousarray(
            cr.transpose(2, 0, 1).reshape(3, -1), np.float32),
        "xy2g": np.ascontiguousarray(
            c2.transpose(2, 0, 1).reshape(3, -1), np.float32),
    }
    emb = np.asarray(w["plang"]["emb"], np.float32)[lang_b]   # [L,64]
    d["xemb"] = np.ascontiguousarray(emb.T, np.float32)
    madd = np.where(lang_b != 0, 0.0, -2e9).astype(np.float32)
    d["madd"] = np.ascontiguousarray(
        np.broadcast_to(madd[None], (128, L)), np.float32)
    return d


def _weights_map(w):
    m = {}
    s1 = w["prp"]["sa1"]
    m["w_r1"], m["b_r1"] = s1[0]["w"][:3], s1[0]["b"][:, None]
    m["w_r2"], m["b_r2"] = s1[1]["w"], s1[1]["b"][:, None]
    m["w_r3"], m["b_r3"] = s1[2]["w"], s1[2]["b"][:, None]
    rg = w["prp"]["glob"]
    m["w_rg1x"], m["w_rg1f"] = rg[0]["w"][:3], rg[0]["w"][3:]
    m["b_rg1"] = rg[0]["b"][:, None]
    m["w_rg2"], m["b_rg2"] = rg[1]["w"], rg[1]["b"][:, None]
    m["w_rg3"], m["b_rg3"] = rg[2]["w"], rg[2]["b"][:, None]
    o1 = w["pop"]["sa1"]
    m["w_s1_1"], m["b_s1_1"] = o1[0]["w"], o1[0]["b"][:, None]
    m["w_s1_2"], m["b_s1_2"] = o1[1]["w"], o1[1]["b"][:, None]
    m["w_s1_3"], m["b_s1_3"] = o1[2]["w"], o1[2]["b"][:, None]
    o2 = w["pop"]["sa2"]
    m["w_s2_1x"], m["w_s2_1f"] = o2[0]["w"][:3], o2[0]["w"][3:]
    m["b_s2_1"] = o2[0]["b"][:, None]
    m["w_s2_2"], m["b_s2_2"] = o2[1]["w"], o2[1]["b"][:, None]
    m["w_s2_3"], m["b_s2_3"] = o2[2]["w"], o2[2]["b"][:, None]
    og = w["pop"]["glob"]
    m["w_g1x"], m["w_g1f"] = og[0]["w"][:3], og[0]["w"][3:]
    m["b_g1"] = og[0]["b"][:, None]
    m["w_g2"], m["b_g2"] = og[1]["w"], og[1]["b"][:, None]
    m["w_g3"], m["b_g3"] = og[2]["w"], og[2]["b"][:, None]
    m["w_lp"], m["b_lp"] = w["plang"]["proj"]["w"], w["plang"]["proj"]["b"][:, None]
    m["w_li"], m["w_lh"] = w["plang"]["wi"], w["plang"]["wh"]
    m["b_l"] = w["plang"]["b"][:, None]
    for i, lpw in enumerate(w["pdg"]["edge"]):
        m[f"w_e{i + 1}"], m[f"b_e{i + 1}"] = lpw["w"], lpw["b"][:, None]
    m["w_f"], m["b_f"] = w["pdg"]["final"]["w"], w["pdg"]["final"]["b"][:, None]
    for i, lpw in enumerate(w["pmlp"]):
        m[f"w_m{i + 1}"], m[f"b_m{i + 1}"] = lpw["w"], lpw["b"][:, None]
    m["w_p1"], m["b_p1"] = w["prel"]["l1"]["w"], w["prel"]["l1"]["b"][:, None]
    m["w_p2"], m["b_p2"] = w["prel"]["l2"]["w"], w["prel"]["l2"]["b"][:, None]
    m["w_q1"], m["b_q1"] = w["pmrel"]["l1"]["w"], w["pmrel"]["l1"]["b"][:, None]
    m["w_q2"], m["b_q2"] = w["pmrel"]["l2"]["w"], w["pmrel"]["l2"]["b"][:, None]
    return {k: np.ascontiguousarray(np.asarray(v, np.float32))
            for k, v in m.items()}



WSHAPES = dict(
    w_r1=(3, 32), b_r1=(32, 1), w_r2=(32, 32), b_r2=(32, 1),
    w_r3=(32, 64), b_r3=(64, 1),
    w_rg1x=(3, 64), w_rg1f=(64, 64), b_rg1=(64, 1),
    w_rg2=(64, 64), b_rg2=(64, 1), w_rg3=(64, 128), b_rg3=(128, 1),
    w_s1_1=(6, 64), b_s1_1=(64, 1), w_s1_2=(64, 64), b_s1_2=(64, 1),
    w_s1_3=(64, 128), b_s1_3=(128, 1),
    w_s2_1x=(3, 128), w_s2_1f=(128, 128), b_s2_1=(128, 1),
    w_s2_2=(128, 128), b_s2_2=(128, 1), w_s2_3=(128, 256), b_s2_3=(256, 1),
    w_g1x=(3, 256), w_g1f=(256, 256), b_g1=(256, 1),
    w_g2=(256, 512), b_g2=(512, 1), w_g3=(512, 128), b_g3=(128, 1),
    w_lp=(64, 64), b_lp=(64, 1), w_li=(64, 512), w_lh=(128, 512),
    b_l=(512, 1),
    w_e1=(512, 128), b_e1=(128, 1), w_e2=(256, 128), b_e2=(128, 1),
    w_e3=(256, 128), b_e3=(128, 1), w_e4=(256, 128), b_e4=(128, 1),
    w_f=(512, 128), b_f=(128, 1),
    w_m1=(128, 128), b_m1=(128, 1), w_m2=(128, 256), b_m2=(256, 1),
    w_m3=(256, 607), b_m3=(607, 1),
    w_p1=(256, 128), b_p1=(128, 1), w_p2=(128, 128), b_p2=(128, 1),
    w_q1=(128, 128), b_q1=(128, 1), w_q2=(128, 128), b_q2=(128, 1),
)


def _wlayout():
    """[(key, chunk_idx, rows, cols, col_off)] packing into [128, TOT]."""
    out, off = [], 0
    for k, (rows, cols) in WSHAPES.items():
        for ci, r0 in enumerate(range(0, rows, 128)):
            r1 = min(rows, r0 + 128)
            out.append((k, ci, r1 - r0, cols, off))
            off += cols
    return out, off


def _pack_weights(wm):
    lay, tot = _wlayout()
    arr = np.zeros((128, tot), np.float32)
    for k, ci, r, c, off in lay:
        arr[:r, off:off + c] = wm[k][ci * 128:ci * 128 + r]
    return arr


# ----------------------------------------------------------------------------
# Device kernel
# ----------------------------------------------------------------------------

def build_bass():
    nc = bacc.Bacc(None)
    ctx = ExitStack()

    def par(name, shape, out=False):
        return nc.declare_dram_parameter(name, list(shape), F32, isOutput=out)

    h0s1 = par("h0s1", (N, 6, 1024))
    xys2 = par("xys2", (N, 3, 512))
    oh2 = par("oh2", (N, 32, 512))
    h0r = par("h0r", (3, N * 64))
    xy1r = par("xy1r", (3, N * NPR))
    xy2g = par("xy2g", (3, N * NP2))
    xemb = par("xemb", (64, L))
    madd = par("madd", (128, L))

    lay, wtot = _wlayout()
    wpack = par("wpack", (128, wtot))
    wshapes = WSHAPES
    objT = par("objT", (607, N), out=True)
    relT = par("relT", (128, NN), out=True)
    mrelT = par("mrelT", (128, NN), out=True)

    with tile.TileContext(nc) as tc:
        cp = ctx.enter_context(tc.tile_pool(name="const", bufs=1))
        wp = ctx.enter_context(tc.tile_pool(name="wts", bufs=1))
        big = ctx.enter_context(tc.tile_pool(name="big", bufs=1))
        lp = ctx.enter_context(tc.tile_pool(name="loop", bufs=3))
        ep = ctx.enter_context(tc.tile_pool(name="ev", bufs=2))
        sp = ctx.enter_context(tc.tile_pool(name="small", bufs=2))
        rp2 = ctx.enter_context(tc.tile_pool(name="rp2", bufs=2))
        rlb = ctx.enter_context(tc.tile_pool(name="rlb", bufs=2))
        pp = ctx.enter_context(tc.tile_pool(name="ps", bufs=7, space="PSUM"))

        def psum(m, n):
            return pp.tile([m, n], F32, tag="ps", name="ps")

        def mm(ps, pairs, rdt=F32R, n0=0):
            # ps: psum AP [M, n<=512]; pairs: K-chunks (lhsT, rhs_full)
            nsz = ps.shape[-1]
            dt = F32  # fp32r needs rounded producers; revisit
            for ki, (lh, rh) in enumerate(pairs):
                nc.tensor.matmul(ps, lh.bitcast(dt),
                                 rh[:, n0:n0 + nsz].bitcast(dt),
                                 start=(ki == 0), stop=(ki == len(pairs) - 1))

        def relu_b(eng, out, src, bcol):
            if eng == "act":
                nc.scalar.activation(out, src, AF.Relu, bias=bcol)
            else:
                nc.vector.tensor_scalar(out, src, bcol, 0.0, AI.add, AI.max)

        def layer(pairs, out_tile, M, Ntot, bcol, mode, nmax=512, rot=0,
                  rdt=F32R):
            # full layer: loop N-chunks: matmul -> evict into out_tile
            # mode: 'relu' | 'bias' | None
            ci = rot
            for n0 in range(0, Ntot, nmax):
                n1 = min(Ntot, n0 + nmax)
                ps = psum(M, n1 - n0)
                mm(ps[:], pairs, n0=n0, rdt=rdt)
                dst = out_tile[:, n0:n1]
                if mode == "relu":
                    relu_b("act" if ci % 2 == 0 else "dve", dst, ps[:], bcol)
                elif mode == "bias":
                    nc.scalar.activation(dst, ps[:], AF.Identity, bias=bcol)
                else:
                    nc.scalar.copy(dst, ps[:])
                ci += 1

        # --- all weights in one packed tile (one DMA, one wait sem)
        WP = wp.tile([128, wtot], F32, tag="wpack", name="WP")
        nc.sync.dma_start(out=WP[:], in_=wpack[:])
        W = {}
        for k, ci, r, c, off in lay:
            ap = WP[0:r, off:off + c]
            if wshapes[k][0] <= 128:
                W[k] = ap
            else:
                W.setdefault(k, []).append(ap)

        from concourse.masks import make_identity
        idt = cp.tile([128, 128], F32, tag="idt")
        make_identity(nc, idt[:])
        iota52 = cp.tile([52, 52], F32, tag="iota")
        nc.gpsimd.iota(iota52[:], pattern=[[1, 52]], base=0,
                       channel_multiplier=0,
                       allow_small_or_imprecise_dtypes=True)
        zero52 = cp.tile([128, 52], F32, tag="z52")
        nc.vector.memset(zero52[:], 0.0)
        ones1 = cp.tile([128, 1], F32, tag="o1")
        nc.vector.memset(ones1[:], 1.0)

        # =========== pn_obj sa1 (per object) ===========
        f1T = big.tile([128, N * NP1], F32, tag="f1T")
        for o in range(N):
            h0 = lp.tile([6, 1024], F32, tag="h0")
            nc.sync.dma_start(out=h0[:], in_=h0s1[o])
            a1 = ep.tile([64, 1024], F32, tag="a1")
            layer([(W["w_s1_1"][:], h0[:])], a1, 64, 1024, W["b_s1_1"][:],
                  "relu", rot=o)
            a2 = ep.tile([64, 1024], F32, tag="a2")
            layer([(W["w_s1_2"][:], a1[:])], a2, 64, 1024, W["b_s1_2"][:],
                  "relu", rot=o + 1)
            for half in range(2):
                ps = psum(128, 512)
                mm(ps[:], [(W["w_s1_3"][:], a2[:])], n0=half * 512)
                nc.vector.tensor_reduce(
                    f1T[:, o * NP1 + half * 16:o * NP1 + (half + 1) * 16],
                    ps[:].rearrange("p (c s) -> p c s", s=NS1),
                    axis=mybir.AxisListType.X, op=AI.max)
            relu_b("act", f1T[:, o * NP1:(o + 1) * NP1],
                   f1T[:, o * NP1:(o + 1) * NP1], W["b_s1_3"][:])

        # =========== pn_obj sa2 (per object) ===========
        f2aT = big.tile([128, N * NP2], F32, tag="f2aT")
        f2bT = big.tile([128, N * NP2], F32, tag="f2bT")
        for o in range(N):
            pst = psum(32, 128)
            nc.tensor.matmul(pst[:], f1T[:, o * NP1:(o + 1) * NP1],
                             idt[:], is_transpose=True, start=True, stop=True)
            f1row = lp.tile([32, 128], F32, tag="f1row")
            nc.scalar.copy(f1row[:], pst[:])
            oh = lp.tile([32, 512], F32, tag="oh")
            nc.sync.dma_start(out=oh[:], in_=oh2[o])
            g2 = ep.tile([128, 512], F32, tag="g2")
            layer([(f1row[:], oh[:])], g2, 128, 512, None, None, rdt=F32)
            x2 = lp.tile([3, 512], F32, tag="x2")
            nc.sync.dma_start(out=x2[:], in_=xys2[o])
            s2a = ep.tile([128, 512], F32, tag="s2a")
            layer([(W["w_s2_1x"][:], x2[:]), (W["w_s2_1f"][:], g2[:])],
                  s2a, 128, 512, W["b_s2_1"][:], "relu", rot=o)
            s2b = ep.tile([128, 512], F32, tag="s2b")
            layer([(W["w_s2_2"][:], s2a[:])], s2b, 128, 512, W["b_s2_2"][:],
                  "relu", rot=o + 1)
            for half in range(2):
                ps = psum(128, 512)
                mm(ps[:], [(W["w_s2_3"][:, half * 128:(half + 1) * 128], s2b[:])])
                dst = (f2aT if half == 0 else f2bT)[:, o * NP2:(o + 1) * NP2]
                nc.vector.tensor_reduce(
                    dst, ps[:].rearrange("p (c s) -> p c s", s=NS2),
                    axis=mybir.AxisListType.X, op=AI.max)
                relu_b("act", dst, dst, W["b_s2_3"][half][:])

        # =========== pn_obj global SA (batched) ===========
        x2g = big.tile([3, N * NP2], F32, tag="x2g")
        nc.sync.dma_start(out=x2g[:], in_=xy2g[:])
        ga = big.tile([128, N * NP2], F32, tag="ga")
        gb = big.tile([128, N * NP2], F32, tag="gb")
        layer([(W["w_g1x"][:, 0:128], x2g[:]),
               (W["w_g1f"][0][:, 0:128], f2aT[:]),
               (W["w_g1f"][1][:, 0:128], f2bT[:])],
              ga, 128, N * NP2, W["b_g1"][0][:], "relu", nmax=416)
        layer([(W["w_g1x"][:, 128:256], x2g[:]),
               (W["w_g1f"][0][:, 128:256], f2aT[:]),
               (W["w_g1f"][1][:, 128:256], f2bT[:])],
              gb, 128, N * NP2, W["b_g1"][1][:], "relu", nmax=416, rot=1)
        gc = []
        for mi in range(4):
            t = big.tile([128, N * NP2], F32, tag=f"gc{mi}")
            layer([(W["w_g2"][0][:, mi * 128:(mi + 1) * 128], ga[:]),
                   (W["w_g2"][1][:, mi * 128:(mi + 1) * 128], gb[:])],
                  t, 128, N * NP2, W["b_g2"][mi][:],
                  "relu", nmax=416, rot=mi)
            gc.append(t)
        ofT = big.tile([128, N], F32, tag="ofT")
        for half in range(2):
            ps = psum(128, 416)
            mm(ps[:], [(W["w_g3"][mi][:], gc[mi][:])
                       for mi in range(4)], n0=half * 416)
            nc.vector.tensor_reduce(
                ofT[:, half * 26:(half + 1) * 26],
                ps[:].rearrange("p (n c) -> p n c", c=NP2),
                axis=mybir.AxisListType.X, op=AI.max)
        relu_b("act", ofT[:], ofT[:], W["b_g3"][:])

        # =========== rel_pn branch (batched) ===========
        h0rt = rp2.tile([3, N * 64], F32, tag="rp", name="h0rt")
        nc.sync.dma_start(out=h0rt[:], in_=h0r[:])
        r1 = rp2.tile([32, N * 64], F32, tag="rp", name="r1")
        layer([(W["w_r1"][:], h0rt[:])], r1, 32, N * 64, W["b_r1"][:], "relu")
        r2 = rp2.tile([32, N * 64], F32, tag="rp", name="r2")
        layer([(W["w_r2"][:], r1[:])], r2, 32, N * 64, W["b_r2"][:], "relu",
              rot=1)
        frT = big.tile([64, N * NPR], F32, tag="frT")
        for qi in range(0, N * 64, 512):
            nsz = min(512, N * 64 - qi)
            ps = psum(64, nsz)
            mm(ps[:], [(W["w_r3"][:], r2[:])], n0=qi)
            nc.vector.tensor_reduce(
                frT[:, qi // NSR:(qi + nsz) // NSR],
                ps[:].rearrange("p (c s) -> p c s", s=NSR),
                axis=mybir.AxisListType.X, op=AI.max)
        relu_b("act", frT[:], frT[:], W["b_r3"][:])

        x1r = big.tile([3, N * NPR], F32, tag="x1r")
        nc.sync.dma_start(out=x1r[:], in_=xy1r[:])
        q1 = big.tile([64, N * NPR], F32, tag="q1")
        layer([(W["w_rg1x"][:], x1r[:]), (W["w_rg1f"][:], frT[:])],
              q1, 64, N * NPR, W["b_rg1"][:], "relu", nmax=416)
        q2 = big.tile([64, N * NPR], F32, tag="q2")
        layer([(W["w_rg2"][:], q1[:])], q2, 64, N * NPR, W["b_rg2"][:],
              "relu", nmax=416, rot=1)
        rpT = big.tile([128, N], F32, tag="rpT")
        ps = psum(128, 416)
        mm(ps[:], [(W["w_rg3"][:], q2[:])])
        nc.vector.tensor_reduce(
            rpT[:], ps[:].rearrange("p (n s) -> p n s", s=NPR),
            axis=mybir.AxisListType.X, op=AI.max)
        relu_b("act", rpT[:], rpT[:], W["b_rg3"][:])

        # =========== LSTM (this core's batch) ===========
        xe = cp.tile([64, L], F32, tag="xe")
        nc.sync.dma_start(out=xe[:], in_=xemb[:])
        mad = cp.tile([128, L], F32, tag="mad")
        nc.sync.dma_start(out=mad[:], in_=madd[:])
        xT = cp.tile([64, L], F32, tag="xT")
        ps = psum(64, L)
        mm(ps[:], [(W["w_lp"][:], xe[:])])
        relu_b("act", xT[:], ps[:], W["b_lp"][:])
        cst = cp.tile([128, 1], F32, tag="cst")
        nc.vector.memset(cst[:], 0.0)
        hs0 = cp.tile([128, 1], F32, tag="hs0")
        nc.vector.memset(hs0[:], 0.0)
        hsT = cp.tile([128, L], F32, tag="hsT")
        hprev = hs0[:]
        for t in range(L):
            psz = psum(128, 4)
            for mi in range(4):
                nc.tensor.matmul(psz[:, mi:mi + 1],
                                 W["w_li"][:, mi * 128:(mi + 1) * 128],
                                 xT[:, t:t + 1], start=True, stop=False)
                nc.tensor.matmul(psz[:, mi:mi + 1],
                                 W["w_lh"][:, mi * 128:(mi + 1) * 128],
                                 hprev, start=False, stop=True)
            gi_ = sp.tile([128, 1], F32, tag="l_i")
            gf_ = sp.tile([128, 1], F32, tag="l_f")
            gg_ = sp.tile([128, 1], F32, tag="l_g")
            go_ = sp.tile([128, 1], F32, tag="l_o")
            nc.scalar.activation(gi_[:], psz[:, 0:1], AF.Sigmoid, bias=W["b_l"][0][:])
            nc.scalar.activation(gf_[:], psz[:, 1:2], AF.Sigmoid, bias=W["b_l"][1][:])
            nc.scalar.activation(gg_[:], psz[:, 2:3], AF.Tanh, bias=W["b_l"][2][:])
            nc.scalar.activation(go_[:], psz[:, 3:4], AF.Sigmoid, bias=W["b_l"][3][:])
            t1 = sp.tile([128, 1], F32, tag="l_t1")
            nc.vector.tensor_tensor(t1[:], gf_[:], cst[:], AI.mult)
            t2 = sp.tile([128, 1], F32, tag="l_t2")
            nc.vector.tensor_tensor(t2[:], gi_[:], gg_[:], AI.mult)
            cst = sp.tile([128, 1], F32, tag="l_c")
            nc.vector.tensor_tensor(cst[:], t1[:], t2[:], AI.add)
            tc_ = sp.tile([128, 1], F32, tag="l_tc")
            nc.scalar.activation(tc_[:], cst[:], AF.Tanh)
            nc.vector.tensor_tensor(hsT[:, t:t + 1], go_[:], tc_[:], AI.mult)
            hprev = hsT[:, t:t + 1]
        hm = cp.tile([128, L], F32, tag="hm")
        nc.vector.tensor_tensor(hm[:], hsT[:], mad[:], AI.add)
        lf = cp.tile([128, 1], F32, tag="lf")
        nc.vector.tensor_reduce(lf[:], hm[:], axis=mybir.AxisListType.X, op=AI.max)
        lfb = big.tile([128, N], F32, tag="lfb")
        nc.vector.tensor_scalar(lfb[:], zero52[:], lf[:], None, AI.add)

        # =========== DGCNN (on-device kNN, k=3) ===========
        onesr = cp.tile([1, N], F32, tag="onesr")
        nc.vector.memset(onesr[:], 1.0)
        mpsq = cp.tile([1, N], F32, tag="mpsq")
        feats = []
        hcur = [rpT, lfb]
        for li in range(4):
            nch = len(hcur)
            pspq = psum(1, N)
            sqs = []
            for c in range(nch):
                sq = sp.tile([128, N], F32, tag=f"sq{c}")
                nc.scalar.activation(sq[:], hcur[c][:], AF.Square)
                sqs.append(sq)
            mm(pspq[:], [(ones1[:], s[:]) for s in sqs], rdt=F32)
            nc.scalar.activation(mpsq[:], pspq[:], AF.Copy, scale=-1.0)
            h2s = []
            for c in range(nch):
                h2 = sp.tile([128, N], F32, tag=f"h2_{c}")
                nc.scalar.activation(h2[:], hcur[c][:], AF.Copy, scale=2.0)
                h2s.append(h2)
            psd2 = psum(52, N)
            mm(psd2[:], [(hcur[c][:], h2s[c][:]) for c in range(nch)]
               + [(mpsq[:], onesr[:]), (onesr[:], mpsq[:])], rdt=F32)
            nd2 = sp.tile([52, N], F32, tag="nd2")
            nc.scalar.copy(nd2[:], psd2[:])
            mx = sp.tile([52, 8], F32, tag="mx")
            nc.vector.max(mx[:], nd2[:])
            mi_ = sp.tile([52, 8], U32, tag="mi")
            nc.vector.max_index(mi_[:], mx[:], nd2[:])
            mif = sp.tile([52, 8], F32, tag="mif")
            nc.vector.tensor_copy(mif[:], mi_[:])

            hrow = []
            for c in range(nch):
                pst = psum(52, 128)
                nc.tensor.matmul(pst[:], hcur[c][:], idt[:],
                                 is_transpose=True, start=True, stop=True)
                hr_ = sp.tile([52, 128], F32, tag=f"hr{c}")
                nc.scalar.copy(hr_[:], pst[:])
                hrow.append(hr_)

            wE = W[f"w_e{li + 1}"]
            acc = sp.tile([128, N], F32, tag="eacc")
            for s in range(3):
                ohT = sp.tile([52, N], F32, tag="ohT")
                nc.vector.tensor_scalar(ohT[:], iota52[:], mif[:, s:s + 1],
                                        None, AI.is_equal)
                psoh = psum(52, N)
                nc.tensor.matmul(psoh[:], ohT[:], idt[0:52, 0:52],
                                 is_transpose=True, start=True, stop=True)
                ohJ = sp.tile([52, N], F32, tag="ohJ")
                nc.scalar.copy(ohJ[:], psoh[:])
                difs = []
                for c in range(nch):
                    psx = psum(128, N)
                    mm(psx[:], [(hrow[c][:], ohJ[:])], rdt=F32)
                    dif = sp.tile([128, N], F32, tag=f"dif{c}")
                    nc.vector.tensor_tensor(dif[:], psx[:], hcur[c][:],
                                            AI.subtract)
                    difs.append(dif)
                pse = psum(128, N)
                pairs = [(wE[c][:], hcur[c][:])
                         for c in range(nch)]
                pairs += [(wE[nch + c][:], difs[c][:])
                          for c in range(nch)]
                mm(pse[:], pairs, rdt=F32)
                if s == 0:
                    nc.scalar.copy(acc[:], pse[:])
                else:
                    nc.vector.tensor_tensor(acc[:], acc[:], pse[:], AI.max)
            hb = sp.tile([128, N], F32, tag="hb")
            nc.scalar.activation(hb[:], acc[:], AF.Identity,
                                 bias=W[f"b_e{li + 1}"][:])
            hn = big.tile([128, N], F32, tag=f"hn{li}")
            nc.vector.scalar_tensor_tensor(hn[:], hb[:], 0.2, hb[:],
                                           AI.mult, AI.max)
            feats.append(hn)
            hcur = [hn]

        gT = big.tile([128, N], F32, tag="gT")
        ps = psum(128, N)
        mm(ps[:], [(W["w_f"][c][:], feats[c][:])
                   for c in range(4)], rdt=F32)
        gb_ = sp.tile([128, N], F32, tag="gb_")
        nc.scalar.activation(gb_[:], ps[:], AF.Identity, bias=W["b_f"][:])
        nc.vector.scalar_tensor_tensor(gT[:], gb_[:], 0.2, gb_[:],
                                       AI.mult, AI.max)

        # =========== relation MLPs ===========
        rf = big.tile([128, N], F32, tag="rf")
        nc.scalar.activation(rf[:], gT[:], AF.Relu)

        rel1r = rlb.tile([128, NN], F32, tag="relb", name="rel1r")
        relor = rlb.tile([128, NN], F32, tag="relb", name="relor")
        mr1r = rlb.tile([128, NN], F32, tag="relb", name="mr1r")

        # layer 1 from on-the-fly expanded (f_i, f_j) chunks
        for bi in range(0, N, 8):
            nb = min(8, N - bi)              # i-blocks in this chunk
            u1 = sp.tile([128, nb * N], F32, tag="u1")
            u2 = sp.tile([128, nb * N], F32, tag="u2")
            for k in range(nb):
                i = bi + k
                nc.vector.tensor_scalar(u1[:, k * N:(k + 1) * N], zero52[:],
                                        rf[:, i:i + 1], None, AI.add)
                nc.scalar.copy(u2[:, k * N:(k + 1) * N], rf[:])
            ps = psum(128, nb * N)
            mm(ps[:], [(W["w_p1"][0][:], u1[:]),
                       (W["w_p1"][1][:], u2[:])])
            relu_b("dve", rel1r[:, bi * N:(bi + nb) * N], ps[:], W["b_p1"][:])

        def rel_layer(wk, bk, src, relu_dst, out_dram):
            for n0 in range(0, NN, 416):
                nsz = min(416, NN - n0)
                ps = psum(128, nsz)
                mm(ps[:], [(W[wk][:], src[:])], n0=n0)
                if relu_dst is not None:
                    relu_b("dve", relu_dst[:, n0:n0 + nsz], ps[:], W[bk][:])
                if out_dram is not None:
                    ot = sp.tile([128, nsz], F32, tag="rel_o")
                    nc.scalar.activation(ot[:], ps[:], AF.Identity,
                                         bias=W[bk][:])
                    nc.sync.dma_start(out=out_dram[:, n0:n0 + nsz], in_=ot[:])

        rel_layer("w_p2", "b_p2", rel1r, relor, relT)
        rel_layer("w_q1", "b_q1", relor, mr1r, None)
        rel_layer("w_q2", "b_q2", mr1r, None, mrelT)

        # =========== object-embedding MLP ===========
        m1 = sp.tile([128, N], F32, tag="m1")
        ps = psum(128, N)
        mm(ps[:], [(W["w_m1"][:], ofT[:])], rdt=F32)
        relu_b("act", m1[:], ps[:], W["b_m1"][:])
        m2t = []
        for mi in range(2):
            ps = psum(128, N)
            mm(ps[:], [(W["w_m2"][:, mi * 128:(mi + 1) * 128], m1[:])], rdt=F32)
            t = sp.tile([128, N], F32, tag=f"m2_{mi}")
            relu_b("act" if mi == 0 else "dve", t[:], ps[:],
                   W["b_m2"][mi][:])
            m2t.append(t)
        for mi in range(5):
            m0, m1c = mi * 128, min(607, (mi + 1) * 128)
            ps = psum(m1c - m0, N)
            mm(ps[:], [(W["w_m3"][0][:, m0:m1c], m2t[0][:]),
                       (W["w_m3"][1][:, m0:m1c], m2t[1][:])], rdt=F32)
            ot = sp.tile([m1c - m0, N], F32, tag="obj_o")
            nc.scalar.activation(ot[:], ps[:], AF.Identity, bias=W["b_m3"][mi][:])
            nc.sync.dma_start(out=objT[m0:m1c, :], in_=ot[:])

        ctx.close()
    if not nc.is_finalized():
        nc.finalize()
    return nc


_NC_CACHE = {}


def _to_np(x):
    if isinstance(x, dict):
        return {k: _to_np(v) for k, v in x.items()}
    if isinstance(x, (list, tuple)):
        return [_to_np(v) for v in x]
    return np.asarray(x)


def kernel(**inputs):
    objects = np.asarray(inputs["objects"], np.float32)
    lang = np.asarray(inputs["lang"])
    w = {k: _to_np(inputs[k]) for k in
         ("prp", "pop", "pdg", "plang", "pmlp", "prel", "pmrel")}
    wm = _weights_map(w)
    wp = _pack_weights(wm)

    in_maps = []
    for b in range(B):
        d = _prep_core(objects[b], lang[b], w)
        d["wpack"] = wp
        in_maps.append(d)

    if "nc" not in _NC_CACHE:
        _NC_CACHE["nc"] = build_bass()
    nc = _NC_CACHE["nc"]
    res = run_bass_kernel_spmd(nc, in_maps, list(range(B))).results

    obj = np.stack([res[b]["objT"].T for b in range(B)], 1)
    rel = np.stack([res[b]["relT"].T.reshape(N, N, 128) for b in range(B)], 2)
    mrel = np.stack([res[b]["mrelT"].T.reshape(N, N, 128) for b in range(B)], 2)
    return obj, rel, mrel
